# revision 1
# baseline (speedup 1.0000x reference)
# Trainium2 Bass kernel for CrossAttentionCacheKVLayer.
#
# Shapes (hardcoded): B=64, Q=16, A=4096, D=128, H=8, HD=16, FF=512.
# Sharding: data-parallel over batch B across 8 NeuronCores (8 outputs/core),
# with per-core source dedup: outputs sharing a KV source (batch_mask
# collisions) are packed into shared "double" slots so e@W_v and the embed DMA
# run once per unique source.  The slot pattern (ndbl doubles + singles) is an
# SPMD-uniform compile parameter derived from batch_mask; the program cache is
# keyed by it.
#
# Host-side restructuring (exact fp32 math):
#   - batch gather kv[batch_mask] -> host gathers embed per output.
#   - LN1(q), gq = LN1(q) @ W_q.T depend only on q -> host.
#   - gq packed block-diagonally (gq_bd[(h,hd),(h,q)]) so all H*Q=128
#     attention rows share full-width 128-contraction matmuls.
#   - K projection folds away: score_T = e @ gkq, gkq = W_k.T @ gq_bd (host;
#     1/sqrt(HD) folded in).
#   - alpha1 -> W_o; alpha2 -> w_ff_out; LN2 affine -> w_ff/b_ff;
#     alpha2*b_ff_out added on host.
#
# Device per core, per source slot s (outputs o in slot):
#   eT[s] (bf16 [D,A]) --PE--> v = e@W_v.T (a-tiles), score_T[o] = e@gkq[o]
#   attn_T[o] = exp(score_T[o])   (ACT, 1024-wide PSUM reads, no max-sub)
#   ctx_ext[o] = attn_T[o].T @ [v|1]  (PE, accum over A; ones col = denoms)
#   ctx_norm = ctx_ext[:, :D]/denom; blockdiag-mask; selector matmul emits
#   transposed per-head context into ctxT_all columns.
# Batched tail over all 8 outputs ([out,q]=128 rows): W_o matmul + residual,
# LN2 (bn_stats, rstd=exp(-0.5 ln(var+eps))), FFN with ACT Silu, residual.

import os
import numpy as np
import ml_dtypes

import concourse.bass as bass
import concourse.mybir as mybir
import concourse.tile as tile
from concourse import bacc
from concourse.bass_utils import run_bass_kernel_spmd

B, Q, A, D, H = 64, 16, 4096, 128, 8
HD = D // H
FF = 512
NCORES = 8
OUTS = B // NCORES    # 8 output batches per core
ATILES = A // 128     # 32
VW = 129              # v tile width: 128 v-dims + ones column
EPS = 1e-5

bf16 = ml_dtypes.bfloat16
f32 = np.float32
dt = mybir.dt
AF = mybir.ActivationFunctionType


def _build_program(with_mask: bool, ndbl: int, repeat: int = 0):
    """ndbl: number of double slots (2 outputs sharing a source) per core.
    Slots: s<ndbl are doubles (outputs 2s, 2s+1); the rest are singles
    (slot s -> output ndbl+s).  repeat>0 wraps the body in an on-device
    For_i loop (timing builds only)."""
    S = OUTS - ndbl  # source slots per core
    SKIP = set(os.environ.get("K_SKIP", "").split(","))
    nc = bacc.Bacc("TRN2", target_bir_lowering=False, debug=False,
                   num_devices=NCORES)

    eT = nc.dram_tensor("eT", [S, D, A], dt.bfloat16, kind="ExternalInput")
    gkq = nc.dram_tensor("gkq", [D, OUTS, D], dt.bfloat16,
                         kind="ExternalInput")
    qres = nc.dram_tensor("qres", [128, D], dt.float32, kind="ExternalInput")
    wvT = nc.dram_tensor("wvT", [D, D], dt.bfloat16, kind="ExternalInput")
    woT = nc.dram_tensor("woT", [D, D], dt.bfloat16, kind="ExternalInput")
    wffT = nc.dram_tensor("wffT", [D, 2 * FF], dt.bfloat16,
                          kind="ExternalInput")
    bff = nc.dram_tensor("bff", [128, 12], dt.float32, kind="ExternalInput")
    woutT = nc.dram_tensor("woutT", [FF, D], dt.bfloat16,
                           kind="ExternalInput")
    selI = nc.dram_tensor("selI", [128, Q], dt.bfloat16,
                          kind="ExternalInput")
    bdmask = nc.dram_tensor("bdmask", [128, 128], dt.bfloat16,
                            kind="ExternalInput")
    maskb = None
    if with_mask:
        maskb = nc.dram_tensor("maskb", [OUTS, A, 128], dt.bfloat16,
                               kind="ExternalInput")
    out_d = nc.dram_tensor("out", [128, D], dt.float32, kind="ExternalOutput")

    def slot_outputs(s):
        return [2 * s, 2 * s + 1] if s < ndbl else [ndbl + s]

    with tile.TileContext(nc) as tc:
        with (
            tc.tile_pool(name="consts", bufs=1) as consts,
            tc.tile_pool(name="persist", bufs=1) as persist,
            tc.tile_pool(name="small", bufs=4) as small,
        ):
            # ---- constants ----
            wv_sb = consts.tile([D, D], dt.bfloat16)
            nc.sync.dma_start(out=wv_sb, in_=wvT[:, :])
            wo_sb = consts.tile([D, D], dt.bfloat16)
            nc.gpsimd.dma_start(out=wo_sb, in_=woT[:, :])
            wff_sb = consts.tile([D, 2 * FF], dt.bfloat16)
            nc.gpsimd.dma_start(out=wff_sb, in_=wffT[:, :])
            bff_sb = consts.tile([128, 12], dt.float32)
            nc.gpsimd.dma_start(out=bff_sb, in_=bff[:, :])
            wout_sb = consts.tile([128, 4, D], dt.bfloat16)
            nc.gpsimd.dma_start(out=wout_sb,
                              in_=woutT[:, :].rearrange("(i p) d -> p i d",
                                                        p=128))
            qres_sb = consts.tile([128, D], dt.float32)
            nc.gpsimd.dma_start(out=qres_sb, in_=qres[:, :])
            gkq_sb = consts.tile([D, OUTS, D], dt.bfloat16)
            nc.sync.dma_start(out=gkq_sb, in_=gkq[:, :, :])
            eps_sb = consts.tile([128, 1], dt.float32)
            nc.vector.memset(eps_sb, EPS)
            selI_sb = consts.tile([128, Q], dt.bfloat16)
            nc.gpsimd.dma_start(out=selI_sb, in_=selI[:, :])
            bdm_sb = consts.tile([128, 128], dt.bfloat16)
            nc.gpsimd.dma_start(out=bdm_sb, in_=bdmask[:, :])

            # accumulated across slots
            ctxT_all = persist.tile([128, 128], dt.bfloat16)
            hidden_sb = persist.tile([128, D], dt.float32)

            import contextlib
            loop_cm = (tc.For_i(0, repeat, 1,
                                hint_engines=(mybir.EngineType.PE,
                                              mybir.EngineType.Activation,
                                              mybir.EngineType.DVE,
                                              mybir.EngineType.SP))
                       if repeat else contextlib.nullcontext())
            with loop_cm:
                _et_b = int(os.environ.get("K_ET", "2"))
                _vx_b = int(os.environ.get("K_VX", "2"))
                _at_b = int(os.environ.get("K_AT", "3"))
                with (
                    tc.tile_pool(name="et", bufs=_et_b) as et_pool,
                    tc.tile_pool(name="vext", bufs=_vx_b) as vext_pool,
                    tc.tile_pool(name="attn", bufs=_at_b) as attn_pool,
                    tc.tile_pool(name="psv", bufs=int(os.environ.get("K_PSV", "2")), space="PSUM") as psv,
                    tc.tile_pool(name="pss", bufs=int(os.environ.get("K_PSS", "2")), space="PSUM") as pss,
                    tc.tile_pool(name="psctx", bufs=int(os.environ.get("K_PSC", "2")), space="PSUM") as psctx,
                ):
                    # iterate outputs; compute v/eT only when the
                    # source changes (paired outputs share the previous one).
                    # Each output's finishing work (last ctx group + softmax
                    # normalize + selector matmul) is deferred into the next
                    # output's score phase so PE never stalls on the
                    # normalize chain.
                    et_sb = None
                    vext_sb = None
                    vext_3d = None
                    tail_a = [None]
                    tail_b = [None]
                    for o in range(OUTS):
                        if o < 2 * ndbl:
                            src_slot, is_new = divmod(o, 2)
                            is_new = (is_new == 0)
                        else:
                            src_slot, is_new = o - ndbl, True
                        if is_new:
                            et_sb = et_pool.tile([D, A], dt.bfloat16,
                                                 tag="et")
                            _nch = int(os.environ.get("K_ETCH", "4"))
                            _mix = os.environ.get("K_ETMIX", "mix") == "mix"
                            for ch in range(_nch):
                                lo = ch * A // _nch
                                hi = (ch + 1) * A // _nch
                                eng = (nc.gpsimd if (_mix and ch % 2)
                                       else nc.sync)
                                eng.dma_start(out=et_sb[:, lo:hi],
                                              in_=eT[src_slot, :, lo:hi])
                            vext_sb = vext_pool.tile([128, ATILES * VW],
                                                     dt.bfloat16, tag="vext")
                            vext_3d = vext_sb[:, :].rearrange(
                                "p (t w) -> p t w", w=VW)
                            nc.vector.memset(vext_3d[:, :, 128], 1.0)

                        if with_mask:
                            mk_sb = et_pool.tile([128, ATILES, 128],
                                                 dt.bfloat16, tag="mask")
                            nc.sync.dma_start(
                                out=mk_sb,
                                in_=maskb[o, :, :].rearrange(
                                    "(t p) h -> p t h", p=128))

                        ps_ctx = psctx.tile([128, 512], dt.float32,
                                            tag="ctx")
                        my_vext = vext_sb

                        def ctx_mms(pa, gg, _ctx=ps_ctx, _vx=vext_sb):
                            if "ctx" in SKIP:
                                return
                            GW = int(os.environ.get("K_GW", "8"))
                            for c in range(GW):
                                tt = GW * gg + c
                                nc.tensor.matmul(
                                    _ctx[:, 0:VW],
                                    lhsT=pa[:, c * 128:(c + 1) * 128],
                                    rhs=_vx[:, tt * VW:tt * VW + VW],
                                    start=(tt == 0), stop=(tt == ATILES - 1))

                        prev = None
                        GW = int(os.environ.get("K_GW", "8"))
                        NG = ATILES // GW
                        for gg in range(NG):     # GW a-tiles per group
                            ps_s = pss.tile([128, GW * 128], dt.float32,
                                            tag="s")
                            ps_v = None
                            if is_new:
                                nv = max(1, GW // 4)
                                ps_v = [psv.tile([128, 512], dt.float32,
                                                 tag="v", name=f"psv{h2}")
                                        for h2 in range(nv)]
                            for c in range(GW):
                                tt = GW * gg + c
                                esl = et_sb[:, tt * 128:(tt + 1) * 128]
                                nc.tensor.matmul(
                                    ps_s[:, c * 128:(c + 1) * 128],
                                    lhsT=esl, rhs=gkq_sb[:, o, :],
                                    start=True, stop=True)
                                if is_new:
                                    pv = ps_v[c // 4]
                                    nc.tensor.matmul(
                                        pv[:, (c % 4) * 128:
                                           (c % 4) * 128 + 128],
                                        lhsT=esl, rhs=wv_sb,
                                        start=True, stop=True)
                            # deferred finishing work of the previous output
                            if gg == 0 and tail_a[0] is not None:
                                tail_a[0]()
                                tail_a[0] = None
                            if gg == min(1, NG - 1) and tail_b[0] is not None:
                                tail_b[0]()
                                tail_b[0] = None
                            if is_new and "vdrain" not in SKIP:
                                for h2 in range(len(ps_v)):
                                    g = (GW // 4) * gg + h2
                                    nc.vector.tensor_copy(
                                        vext_3d[:, 4 * g:4 * g + 4, 0:128],
                                        ps_v[h2][:, :].rearrange(
                                            "p (t w) -> p t w", w=128))
                            if with_mask:
                                nc.vector.tensor_add(
                                    ps_s[:, :], ps_s[:, :],
                                    mk_sb[:, GW * gg:GW * gg + GW, :]
                                    .rearrange("p t h -> p (t h)"))
                            at = attn_pool.tile([128, GW * 128], dt.bfloat16,
                                                tag="at")
                            if "exp" not in SKIP:
                                nc.scalar.activation(at, ps_s, AF.Exp)
                            if prev is not None:
                                ctx_mms(*prev)
                            prev = (at, gg)

                        def make_tail(o=o, prev=prev, ps_ctx=ps_ctx,
                                      ctx_mms=ctx_mms):
                            def ta():
                                ctx_mms(*prev)
                            def tb():
                                if "norm" in SKIP:
                                    return
                                recip = small.tile([128, 1], dt.float32,
                                                   tag="recip")
                                nc.vector.reciprocal(recip,
                                                     ps_ctx[:, 128:129])
                                ctxn = small.tile([128, D], dt.bfloat16,
                                                  tag="ctxn")
                                nc.scalar.activation(ctxn, ps_ctx[:, 0:D],
                                                     AF.Copy, scale=recip)
                                ctxm = small.tile([128, D], dt.bfloat16,
                                                  tag="ctxm")
                                nc.vector.tensor_mul(ctxm, ctxn, bdm_sb)
                                nc.tensor.matmul(ps_ctx[:, 256:256 + Q],
                                                 lhsT=ctxm, rhs=selI_sb,
                                                 start=True, stop=True)
                                nc.vector.tensor_copy(
                                    ctxT_all[:, o * Q:(o + 1) * Q],
                                    ps_ctx[:, 256:256 + Q])
                            return ta, tb
                        tail_a[0], tail_b[0] = make_tail()
                    tail_a[0]()
                    tail_b[0]()

                # ---- batched tail: rows are (out, q) = 128 ----
                def _tail():
                    with (
                      tc.tile_pool(name="ps_ao", bufs=1, space="PSUM") as pao,
                      tc.tile_pool(name="ps_ffa", bufs=2, space="PSUM") as pfa,
                      tc.tile_pool(name="ps_ffb", bufs=2, space="PSUM") as pfb,
                      tc.tile_pool(name="ps_ffo", bufs=1, space="PSUM") as pfo,
                      tc.tile_pool(name="ps_tr", bufs=1, space="PSUM") as ptr,
                  ):
                      ps_ao = pao.tile([128, D], dt.float32)
                      nc.tensor.matmul(ps_ao, lhsT=ctxT_all, rhs=wo_sb,
                                       start=True, stop=True)
                      nc.vector.tensor_add(hidden_sb, qres_sb, ps_ao)

                      stats = small.tile([128, 6], dt.float32, tag="st")
                      nc.vector.bn_stats(out=stats, in_=hidden_sb)
                      mv = small.tile([128, 2], dt.float32, tag="mv")
                      nc.vector.bn_aggr(out=mv, in_=stats)
                      if os.environ.get("K_RSQRT", "ln") == "ln":
                          lnv = small.tile([128, 1], dt.float32, tag="lnv")
                          nc.scalar.activation(lnv, mv[:, 1:2], AF.Ln,
                                               bias=eps_sb)
                          rstd = small.tile([128, 1], dt.float32, tag="rstd")
                          nc.scalar.activation(rstd, lnv, AF.Exp, scale=-0.5)
                          return_rstd = rstd
                      # rstd = 1/sqrt(var+eps): quake seed + 2 Newton steps
                      ve = small.tile([128, 1], dt.float32, tag="ve")
                      if os.environ.get("K_RSQRT", "ln") == "quake":
                        nc.vector.tensor_scalar_add(ve, mv[:, 1:2], EPS)
                        vh = small.tile([128, 1], dt.float32, tag="vh")
                        nc.vector.tensor_scalar_mul(vh, ve, 0.5)
                        y0 = small.tile([128, 1], dt.float32, tag="y0")
                        nc.vector.tensor_scalar(
                            y0.bitcast(dt.int32), ve.bitcast(dt.int32),
                            1, None, mybir.AluOpType.logical_shift_right)
                        nc.vector.tensor_scalar(
                            y0.bitcast(dt.int32), y0.bitcast(dt.int32),
                            -1, 0x5f3759df, mybir.AluOpType.mult,
                            mybir.AluOpType.add)
                        rstd = small.tile([128, 1], dt.float32, tag="rstd")
                        for _it in range(2):
                            t1 = small.tile([128, 1], dt.float32,
                                            name=f"nt{_it}", tag="nt")
                            nc.vector.tensor_mul(t1, y0, y0)
                            nc.vector.tensor_mul(t1, t1, vh)
                            nc.vector.tensor_scalar(
                                t1, t1, -1.0, 1.5, mybir.AluOpType.mult,
                                mybir.AluOpType.add)
                            dst = rstd if _it == 1 else y0
                            nc.vector.tensor_mul(dst, y0, t1)
                      else:
                        rstd = return_rstd
                      nbias = small.tile([128, 1], dt.float32, tag="nb")
                      nc.vector.tensor_mul(nbias, mv[:, 0:1], rstd)
                      nbias2 = small.tile([128, 1], dt.float32, tag="nb2")
                      nc.vector.tensor_scalar_mul(nbias2, nbias, -1.0)
                      hn = small.tile([128, D], dt.bfloat16, tag="hn")
                      nc.scalar.activation(hn, hidden_sb, AF.Identity,
                                           bias=nbias2, scale=rstd)
                      # transpose hn -> [dIn, (out,q)] via identity matmul
                      ps_hnT = ptr.tile([128, 128], dt.bfloat16)
                      nc.tensor.matmul(ps_hnT, lhsT=hn, rhs=bdm_idn(nc, consts),
                                       is_transpose=True, start=True, stop=True)
                      hnT = small.tile([128, 128], dt.bfloat16, tag="hnT")
                      nc.vector.tensor_copy(hnT, ps_hnT)

                      # FFN: ab_T chunks [ff(128), (out,q)(128)]; a-chunk i
                      # pairs with b-chunk i+4.  bff cols 0..7 = +bias chunks.
                      hT = [None] * 4
                      for i in range(4):
                          ps_a = pfa.tile([128, 128], dt.float32, tag="ffa")
                          ps_b = pfb.tile([128, 128], dt.float32, tag="ffb")
                          nc.tensor.matmul(
                              ps_a, lhsT=wff_sb[:, i * 128:(i + 1) * 128],
                              rhs=hnT, start=True, stop=True)
                          nc.tensor.matmul(
                              ps_b, lhsT=wff_sb[:, (i + 4) * 128:(i + 5) * 128],
                              rhs=hnT, start=True, stop=True)
                          sa = small.tile([128, 128], dt.float32, tag="sa")
                          nc.scalar.activation(sa, ps_a, AF.Silu,
                                               bias=bff_sb[:, i:i + 1])
                          ub = small.tile([128, 128], dt.float32, tag="ub")
                          nc.vector.tensor_scalar_add(ub, ps_b,
                                                      bff_sb[:, i + 4:i + 5])
                          hT_i = small.tile([128, 128], dt.bfloat16,
                                            tag=f"hT{i}")
                          nc.vector.tensor_mul(hT_i, sa, ub)
                          hT[i] = hT_i

                      ps_ff = pfo.tile([128, D], dt.float32)
                      for i in range(4):
                          nc.tensor.matmul(ps_ff, lhsT=hT[i],
                                           rhs=wout_sb[:, i, :],
                                           start=(i == 0), stop=(i == 3))
                      out_sb = small.tile([128, D], dt.float32, tag="out")
                      nc.vector.tensor_add(out_sb, hidden_sb, ps_ff)
                      nc.sync.dma_start(out=out_d[:, :], in_=out_sb)
                if "tail" not in SKIP:
                    _tail()

    nc.compile()
    return nc


_IDENT = {}


def bdm_idn(nc, consts):
    # lazily-created bf16 identity for the hn transpose
    key = id(nc)
    if key not in _IDENT:
        from concourse.masks import make_identity
        ident = consts.tile([128, 128], dt.bfloat16)
        make_identity(nc, ident)
        _IDENT[key] = ident
    return _IDENT[key]


_PROG_CACHE: dict = {}


def _get_program(key):
    if key not in _PROG_CACHE:
        _PROG_CACHE[key] = _build_program(*key)
    return _PROG_CACHE[key]


def _layernorm_np(x, g, b, eps=1e-5):
    mu = x.mean(axis=-1, keepdims=True)
    var = x.var(axis=-1, keepdims=True)
    return (x - mu) / np.sqrt(var + eps) * g + b


def _pack(batch_mask):
    """Group outputs by source; return (ndbl, cores) where cores[c] =
    (sources list, outputs list in device order)."""
    from collections import defaultdict
    groups = defaultdict(list)
    for b, s in enumerate(batch_mask.tolist()):
        groups[int(s)].append(b)
    doubles, singles = [], []
    for src, bs in groups.items():
        i = 0
        while i + 1 < len(bs):
            doubles.append((src, bs[i], bs[i + 1]))
            i += 2
        if i < len(bs):
            singles.append((src, bs[i]))
    ndbl = min(2, len(doubles) // NCORES)
    need = NCORES * ndbl
    for src, b1, b2 in doubles[need:]:
        singles += [(src, b1), (src, b2)]
    doubles = doubles[:need]
    nsng = OUTS - 2 * ndbl
    cores = []
    for c in range(NCORES):
        dbl = doubles[c * ndbl:(c + 1) * ndbl]
        sng = singles[c * nsng:(c + 1) * nsng]
        sources = [d[0] for d in dbl] + [s[0] for s in sng]
        outputs = []
        for d in dbl:
            outputs += [d[1], d[2]]
        outputs += [s[1] for s in sng]
        cores.append((sources, outputs))
    return ndbl, cores


def prepare_in_maps(q, embed, attn_mask, batch_mask, W_kv, W_q, W_o,
                    ln1_g, ln1_b, ln2_g, ln2_b, alpha1, alpha2,
                    w_ff, b_ff, w_ff_out, b_ff_out):
    q = np.asarray(q, f32)
    embed = np.asarray(embed, f32)
    attn_mask = np.asarray(attn_mask)
    batch_mask = np.asarray(batch_mask)
    W_kv = np.asarray(W_kv, f32)
    W_q = np.asarray(W_q, f32)
    W_o = np.asarray(W_o, f32)
    a1 = float(np.asarray(alpha1).reshape(-1)[0])
    a2 = float(np.asarray(alpha2).reshape(-1)[0])
    w_ff = np.asarray(w_ff, f32)
    b_ff = np.asarray(b_ff, f32)
    w_ff_out = np.asarray(w_ff_out, f32)
    b_ff_out = np.asarray(b_ff_out, f32)
    ln1_g = np.asarray(ln1_g, f32)
    ln1_b = np.asarray(ln1_b, f32)
    ln2_g = np.asarray(ln2_g, f32)
    ln2_b = np.asarray(ln2_b, f32)

    W_k = W_kv[:D, :]
    W_v = W_kv[D:, :]

    q_norm = _layernorm_np(q, ln1_g, ln1_b)             # [B, Q, D]
    gq = (q_norm @ W_q.T) / np.sqrt(np.float32(HD))     # [B, Q, D]
    gq_bd = np.zeros((B, D, D), f32)
    gqr = gq.reshape(B, Q, H, HD)
    for h in range(H):
        gq_bd[:, h * HD:(h + 1) * HD, h * Q:(h + 1) * Q] = \
            gqr[:, :, h, :].transpose(0, 2, 1)
    gkq = np.einsum('dk,bkh->bdh', W_k.T, gq_bd)        # [B, D, 128]

    wvT_h = np.ascontiguousarray(W_v.T).astype(bf16)
    woT_h = np.ascontiguousarray((a1 * W_o).T).astype(bf16)
    wffT_h = np.ascontiguousarray((w_ff * ln2_g[None, :]).T).astype(bf16)
    bff_eff = b_ff + w_ff @ ln2_b
    bff_h = np.zeros((128, 12), f32)
    bff_h[:, 0:8] = bff_eff.reshape(8, 128).T
    bff_h[:, 8:12] = 0.5 * bff_eff.reshape(8, 128).T[:, 0:4]
    woutT_h = np.ascontiguousarray((a2 * w_ff_out).T).astype(bf16)

    selI_h = np.tile(np.eye(Q, dtype=f32), (H, 1)).astype(bf16)
    bdm = np.zeros((128, 128), f32)
    for h in range(H):
        bdm[h * Q:(h + 1) * Q, h * HD:(h + 1) * HD] = 1.0
    bdm_h = bdm.astype(bf16)

    with_mask = bool(attn_mask.any())
    ndbl, cores = _pack(batch_mask)

    in_maps = []
    perm = []
    for c in range(NCORES):
        sources, outputs = cores[c]
        perm.append(outputs)
        eT_c = np.ascontiguousarray(
            embed[sources].transpose(0, 2, 1)).astype(bf16)   # [S, D, A]
        m = {
            "eT": eT_c,
            "gkq": np.ascontiguousarray(
                gkq[outputs].transpose(1, 0, 2)).astype(bf16),
            "qres": np.ascontiguousarray(q[outputs].reshape(OUTS * Q, D)),
            "wvT": wvT_h,
            "woT": woT_h,
            "wffT": wffT_h,
            "bff": bff_h,
            "woutT": woutT_h,
            "selI": selI_h,
            "bdmask": bdm_h,
        }
        if with_mask:
            mb = np.where(attn_mask[outputs], np.float32(-30.0),
                          np.float32(0.0))                 # [OUTS, Q, A]
            m["maskb"] = np.ascontiguousarray(
                np.tile(mb.transpose(0, 2, 1), (1, 1, H))).astype(bf16)
        in_maps.append(m)
    post_add = a2 * b_ff_out
    return in_maps, (with_mask, ndbl), post_add, perm


def assemble_output(results, post_add, perm):
    out = np.empty((B, Q, D), f32)
    for c in range(NCORES):
        o = results[c]["out"].reshape(OUTS, Q, D)
        for j, gb in enumerate(perm[c]):
            out[gb] = o[j]
    if post_add is not None and np.any(post_add):
        out = out + post_add[None, None, :].astype(f32)
    return out


def kernel(**inputs):
    in_maps, key, post_add, perm = prepare_in_maps(**inputs)
    nc = _get_program(key)
    res = run_bass_kernel_spmd(nc, in_maps, core_ids=list(range(NCORES)))
    return assemble_output(res.results, post_add, perm)

